# revision 5
# baseline (speedup 1.0000x reference)
"""Causal multi-head attention (B=4, S=2048, D=2048, H=16, RoPE) on 8 TRN2 NeuronCores.

Sharding: core c handles (batch b = c//2, head-group g = c%2) -- 8 heads per core.
Each core computes its head-group's Q/K/V projections (column-sharded weights),
RoPE, causal softmax attention, and the row-sharded Wo partial product.
The host sums the two partial outputs per batch (the "all-reduce") and
transposes back.

Device-side layout is fully transposed ("feature-major"): activations are kept
as [feature, seq] so every GEMM contracts over the partition dimension without
any on-device transposes. All matmul inputs are bf16 (fp32 accumulate in PSUM);
softmax runs in fp32.
"""

import math
import sys
import types

import numpy as np
import ml_dtypes

BF16 = ml_dtypes.bfloat16

S = 2048
D = 2048
H = 16
DK = 128
B = 4
E = 1024          # head-group width (8 heads x 128)
HPC = 8           # heads per core
NT_D = 16         # 128-wide tiles along the contraction (model) dim
NT_S4 = 4         # 512-wide tiles along seq
NT_S16 = 16       # 128-wide tiles along seq
ROPE_THETA = 10000.0

# Set by test harness to capture a profile; kernel() then stores results here.
TRACE = False
LAST_RESULT = None

_PROGRAM_CACHE = {}


def _install_ntff_hook():
    """Register the NTFF profile hook that this image's antenv lacks.

    Only needed when TRACE=True; degrades silently if the axon .so predates
    NRT profiling.
    """
    if "antenv.axon_hooks" in sys.modules:
        return
    holder = {"hook": None}
    mod = types.ModuleType("antenv.axon_hooks")
    mod.set_axon_ntff_profile_hook = lambda h: holder.__setitem__("hook", h)
    mod.get_axon_ntff_profile_hook = lambda: holder["hook"]
    sys.modules["antenv.axon_hooks"] = mod
    try:
        from trn_agent_boot.trn_boot import _ntff_profile_via_ctypes

        mod.set_axon_ntff_profile_hook(
            _ntff_profile_via_ctypes("/opt/axon/libaxon_pjrt.so")
        )
    except Exception:
        pass


def _build_program():
    """Build + compile the single-core Bass program (same program on all 8 cores)."""
    if "nc" in _PROGRAM_CACHE:
        return _PROGRAM_CACHE["nc"]

    from contextlib import ExitStack

    import concourse.mybir as mybir
    import concourse.tile as tile
    from concourse import bacc

    F32 = mybir.dt.float32
    B16 = mybir.dt.bfloat16

    nc = bacc.Bacc("TRN2", target_bir_lowering=False, debug=False, num_devices=8)

    xt = nc.dram_tensor("xt", [NT_D, 128, S], B16, kind="ExternalInput").ap()
    wq = nc.dram_tensor("wq", [HPC, NT_D, 128, 128], B16, kind="ExternalInput").ap()
    wk = nc.dram_tensor("wk", [HPC, NT_D, 128, 128], B16, kind="ExternalInput").ap()
    wv = nc.dram_tensor("wv", [NT_D, 128, E], B16, kind="ExternalInput").ap()
    wo = nc.dram_tensor("wo", [NT_D, HPC, 128, 128], B16, kind="ExternalInput").ap()
    cos = nc.dram_tensor("cos", [128, S], B16, kind="ExternalInput").ap()
    sin = nc.dram_tensor("sin", [128, S], B16, kind="ExternalInput").ap()
    msk = nc.dram_tensor("msk", [4, 128, 512], B16, kind="ExternalInput").ap()
    ones = nc.dram_tensor("ones", [128, 1], B16, kind="ExternalInput").ap()
    outt = nc.dram_tensor("outt", [D, S], F32, kind="ExternalOutput").ap()

    Exp = mybir.ActivationFunctionType.Exp

    with tile.TileContext(nc) as tc, ExitStack() as ctx:
        cpool = ctx.enter_context(tc.tile_pool(name="const", bufs=1))
        cos_t = cpool.tile([128, S], B16, tag="cos", name="cos_t")
        nc.sync.dma_start(out=cos_t, in_=cos)
        sin_t = cpool.tile([128, S], B16, tag="sin", name="sin_t")
        nc.sync.dma_start(out=sin_t, in_=sin)
        msk_t = cpool.tile([128, 4 * 512], B16, tag="msk", name="msk_t")
        nc.sync.dma_start(
            out=msk_t.rearrange("p (m f) -> p m f", m=4),
            in_=msk.rearrange("m p f -> p m f"),
        )
        one_t = cpool.tile([128, 1], B16, tag="one", name="one_t")
        nc.sync.dma_start(out=one_t, in_=ones)

        # Persistent activation stores (bf16): Q^T/K^T per head, V natural.
        qk_pool = ctx.enter_context(tc.tile_pool(name="qk", bufs=HPC))
        v_pool = ctx.enter_context(tc.tile_pool(name="v", bufs=NT_S16))
        qts, kts, vts = [], [], []

        # ---------------- Phase 1: projections ----------------
        with ExitStack() as p1ctx:
            xt_pool = p1ctx.enter_context(tc.tile_pool(name="xt", bufs=NT_D))
            xts = []
            for d in range(NT_D):
                xtile = xt_pool.tile([128, S], B16, tag="xt", name=f"xt_{d}")
                nc.sync.dma_start(out=xtile, in_=xt[d])
                xts.append(xtile)

            p1ps = p1ctx.enter_context(
                tc.tile_pool(name="p1ps", bufs=8, space="PSUM")
            )

            # V projection: out[s,dv] with X^T tiles stationary, WvT moving.
            with tc.tile_pool(name="wvp", bufs=NT_D) as wv_pool:
                wvts = []
                for d in range(NT_D):
                    wvt = wv_pool.tile([128, E], B16, tag="wv", name=f"wv_{d}")
                    nc.sync.dma_start(out=wvt, in_=wv[d])
                    wvts.append(wvt)
                for s in range(NT_S16):
                    pv0 = p1ps.tile([128, 512], F32, tag="p1", name=f"pv0_{s}")
                    pv1 = p1ps.tile([128, 512], F32, tag="p1", name=f"pv1_{s}")
                    for d in range(NT_D):
                        lhsT = xts[d][:, s * 128 : (s + 1) * 128]
                        nc.tensor.matmul(
                            pv0, lhsT=lhsT, rhs=wvts[d][:, 0:512],
                            start=(d == 0), stop=(d == NT_D - 1),
                        )
                        nc.tensor.matmul(
                            pv1, lhsT=lhsT, rhs=wvts[d][:, 512:1024],
                            start=(d == 0), stop=(d == NT_D - 1),
                        )
                    vt = v_pool.tile([128, E], B16, tag="vt", name=f"vt_{s}")
                    nc.vector.tensor_copy(out=vt[:, 0:512], in_=pv0)
                    nc.vector.tensor_copy(out=vt[:, 512:1024], in_=pv1)
                    vts.append(vt)

            # Q^T / K^T projections (weights stationary) + fused RoPE epilogue.
            with tc.tile_pool(name="wst", bufs=2) as w_pool, tc.tile_pool(
                name="rope", bufs=2
            ) as r_pool:
                for wdram, outlist, nm in ((wq, qts, "q"), (wk, kts, "k")):
                    for e in range(HPC):
                        wt = w_pool.tile([128, NT_D * 128], B16, tag="w",
                                         name=f"w{nm}_{e}")
                        nc.sync.dma_start(
                            out=wt.rearrange("p (d f) -> p d f", d=NT_D),
                            in_=wdram[e].rearrange("d p f -> p d f"),
                        )
                        psums = [
                            p1ps.tile([128, 512], F32, tag="p1",
                                      name=f"p{nm}_{e}_{s4}")
                            for s4 in range(NT_S4)
                        ]
                        for d in range(NT_D):
                            lhsT = wt[:, d * 128 : (d + 1) * 128]
                            for s4 in range(NT_S4):
                                nc.tensor.matmul(
                                    psums[s4], lhsT=lhsT,
                                    rhs=xts[d][:, s4 * 512 : (s4 + 1) * 512],
                                    start=(d == 0), stop=(d == NT_D - 1),
                                )
                        qh = qk_pool.tile([128, S], B16, tag=nm + "t",
                                          name=f"{nm}h_{e}")
                        for s4 in range(NT_S4):
                            sl = slice(s4 * 512, (s4 + 1) * 512)
                            ps = psums[s4]
                            t1 = r_pool.tile([128, 512], F32, tag="t1",
                                             name=f"t1_{nm}_{e}_{s4}")
                            u = r_pool.tile([128, 512], F32, tag="u",
                                            name=f"u_{nm}_{e}_{s4}")
                            nc.vector.tensor_mul(out=t1, in0=ps, in1=cos_t[:, sl])
                            nc.vector.tensor_mul(
                                out=u[0:64, :], in0=ps[64:128, :],
                                in1=sin_t[0:64, sl],
                            )
                            nc.vector.tensor_mul(
                                out=u[64:128, :], in0=ps[0:64, :],
                                in1=sin_t[64:128, sl],
                            )
                            nc.vector.tensor_add(out=qh[:, sl], in0=t1, in1=u)
                        outlist.append(qh)

        # ---------------- Phase 2: causal attention per head ----------------
        attn_pool = ctx.enter_context(tc.tile_pool(name="attn", bufs=HPC))
        ats = []
        with tc.tile_pool(name="pt", bufs=3) as pt_pool, tc.tile_pool(
            name="rcp", bufs=2
        ) as rc_pool, tc.tile_pool(name="aps", bufs=1, space="PSUM") as aps:
            for h in range(HPC):
                at = attn_pool.tile([128, S], B16, tag="at", name=f"at_{h}")
                for s4 in range(NT_S4):
                    nsk = 4 * s4 + 4
                    sl = slice(s4 * 512, (s4 + 1) * 512)
                    pat = aps.tile([128, 512], F32, tag="pat", bufs=2,
                                   name=f"pat_{h}_{s4}")
                    pde = aps.tile([1, 512], F32, tag="pde", bufs=2,
                                   name=f"pde_{h}_{s4}")
                    for sk in range(nsk):
                        psc = aps.tile([128, 512], F32, tag="psc", bufs=3,
                                       name=f"psc_{h}_{s4}_{sk}")
                        nc.tensor.matmul(
                            psc,
                            lhsT=kts[h][:, sk * 128 : (sk + 1) * 128],
                            rhs=qts[h][:, sl],
                            start=True, stop=True,
                        )
                        pt = pt_pool.tile([128, 512], B16, tag="pt",
                                          name=f"pt_{h}_{s4}_{sk}")
                        nc.scalar.activation(out=pt, in_=psc, func=Exp)
                        r = sk - 4 * s4
                        if r >= 0:
                            nc.vector.tensor_mul(
                                out=pt, in0=pt,
                                in1=msk_t[:, r * 512 : (r + 1) * 512],
                            )
                        nc.tensor.matmul(
                            pat, lhsT=vts[sk][:, h * 128 : (h + 1) * 128],
                            rhs=pt, start=(sk == 0), stop=(sk == nsk - 1),
                        )
                        nc.tensor.matmul(
                            pde, lhsT=one_t, rhs=pt,
                            start=(sk == 0), stop=(sk == nsk - 1),
                        )
                    rc = rc_pool.tile([1, 512], F32, tag="rc",
                                      name=f"rc_{h}_{s4}")
                    nc.vector.reciprocal(out=rc, in_=pde)
                    rcb = rc_pool.tile([128, 512], F32, tag="rcb",
                                       name=f"rcb_{h}_{s4}")
                    nc.gpsimd.partition_broadcast(out_ap=rcb, in_ap=rc)
                    nc.vector.tensor_mul(out=at[:, sl], in0=pat, in1=rcb)
                ats.append(at)

        # ---------------- Phase 3: Wo partial product ----------------
        with tc.tile_pool(name="wop", bufs=2) as wo_pool, tc.tile_pool(
            name="outp", bufs=4
        ) as out_pool, tc.tile_pool(name="wops", bufs=8, space="PSUM") as wops:
            for eo in range(NT_D):
                wot = wo_pool.tile([128, HPC * 128], B16, tag="wo",
                                   name=f"wo_{eo}")
                nc.sync.dma_start(
                    out=wot.rearrange("p (d f) -> p d f", d=HPC),
                    in_=wo[eo].rearrange("d p f -> p d f"),
                )
                psums = [
                    wops.tile([128, 512], F32, tag="pwo", name=f"pwo_{eo}_{s4}")
                    for s4 in range(NT_S4)
                ]
                for hv in range(HPC):
                    lhsT = wot[:, hv * 128 : (hv + 1) * 128]
                    for s4 in range(NT_S4):
                        nc.tensor.matmul(
                            psums[s4], lhsT=lhsT,
                            rhs=ats[hv][:, s4 * 512 : (s4 + 1) * 512],
                            start=(hv == 0), stop=(hv == HPC - 1),
                        )
                for s4 in range(NT_S4):
                    ot = out_pool.tile([128, 512], F32, tag="ot",
                                       name=f"ot_{eo}_{s4}")
                    nc.vector.tensor_copy(out=ot, in_=psums[s4])
                    nc.sync.dma_start(
                        out=outt[eo * 128 : (eo + 1) * 128,
                                 s4 * 512 : (s4 + 1) * 512],
                        in_=ot,
                    )

    nc.compile()
    _PROGRAM_CACHE["nc"] = nc
    return nc


def _host_prep(x, Wq, Wk, Wv, Wo):
    """Shard + lay out inputs for the 8 cores. Returns list of in_maps."""
    # Within-head permutation: [even dk indices, odd dk indices] so the RoPE
    # pair (2i, 2i+1) becomes (row i, row 64+i) of each head's 128-row block.
    perm1 = np.concatenate([np.arange(0, DK, 2), np.arange(1, DK, 2)])
    perm = np.concatenate([h * DK + perm1 for h in range(H)])

    scale = 1.0 / math.sqrt(DK)
    WqP = (Wq * scale)[perm]          # fold 1/sqrt(dk) into Q
    WkP = Wk[perm]

    # RoPE tables in the permuted feature-major layout [128, S].
    inv_freq = 1.0 / (ROPE_THETA ** (np.arange(0, DK, 2, dtype=np.float64) / DK))
    ang = inv_freq[:, None] * np.arange(S, dtype=np.float64)[None, :]  # [64, S]
    cosP = np.vstack([np.cos(ang), np.cos(ang)]).astype(BF16)
    sinP = np.vstack([-np.sin(ang), np.sin(ang)]).astype(BF16)

    # Causal 0/1 masks for the 4 diagonal-tile offsets: valid iff 128r+i <= j.
    i_idx = np.arange(128)[None, :, None]
    j_idx = np.arange(512)[None, None, :]
    r_idx = np.arange(4)[:, None, None]
    masks = ((128 * r_idx + i_idx) <= j_idx).astype(BF16)  # [4, 128, 512]

    ones = np.ones((128, 1), dtype=BF16)

    def lhsT_blocks(Wt, n_out_tiles):
        # Wt: [contraction, width] (feature-major).
        # -> [n_out_tiles, contraction//128, 128, 128] blocked lhsT tiles.
        kt = Wt.shape[0] // 128
        width = Wt.shape[1]
        blk = Wt.reshape(kt, 128, n_out_tiles, width // n_out_tiles)
        return np.ascontiguousarray(blk.transpose(2, 0, 1, 3)).astype(BF16)

    per_group = []
    for g in range(2):
        rows = slice(g * E, (g + 1) * E)
        wq_b = lhsT_blocks(WqP[rows].T, HPC)
        wk_b = lhsT_blocks(WkP[rows].T, HPC)
        wv_b = np.ascontiguousarray(
            Wv[rows].T.reshape(NT_D, 128, E)
        ).astype(BF16)
        # WoT [E, D]: lhsT blocks are [dv, e_out] tiles.
        wo_b = lhsT_blocks(np.ascontiguousarray(Wo[:, rows].T), NT_D)
        per_group.append((wq_b, wk_b, wv_b, wo_b))

    xts = []
    for b in range(B):
        xts.append(
            np.ascontiguousarray(x[b].T).astype(BF16).reshape(NT_D, 128, S)
        )

    in_maps = []
    for c in range(8):
        b, g = c // 2, c % 2
        wq_b, wk_b, wv_b, wo_b = per_group[g]
        in_maps.append(
            {
                "xt": xts[b],
                "wq": wq_b,
                "wk": wk_b,
                "wv": wv_b,
                "wo": wo_b,
                "cos": cosP,
                "sin": sinP,
                "msk": masks,
                "ones": ones,
            }
        )
    return in_maps


def kernel(x, Wq, Wk, Wv, Wo):
    global LAST_RESULT
    x = np.asarray(x, dtype=np.float32)
    Wq = np.asarray(Wq, dtype=np.float32)
    Wk = np.asarray(Wk, dtype=np.float32)
    Wv = np.asarray(Wv, dtype=np.float32)
    Wo = np.asarray(Wo, dtype=np.float32)

    if TRACE:
        _install_ntff_hook()

    from concourse.bass_utils import run_bass_kernel_spmd

    nc = _build_program()
    in_maps = _host_prep(x, Wq, Wk, Wv, Wo)
    res = run_bass_kernel_spmd(nc, in_maps, list(range(8)), trace=TRACE)
    LAST_RESULT = res

    out = np.empty((B, S, D), dtype=np.float32)
    for b in range(B):
        part = res.results[2 * b]["outt"] + res.results[2 * b + 1]["outt"]
        out[b] = part.T
    return out
